# revision 11
# baseline (speedup 1.0000x reference)
"""Trainium2 Bass kernel for MultiHeadCrossAttention.

Problem: y = proj(softmax(mask(q @ k^T / sqrt(Dh))) @ v) with
  x: (16, 1024, 1024) f32, cond: (16, 120, 1024) f32, mask: (16, 120) i32,
  Wq: (1024, 1024), Wkv: (2048, 1024), Wp: (1024, 1024); H=16 heads, Dh=64.
  Biases are all zeros per the problem spec and are skipped.

Sharding: pure data-parallel over batch B=16 -> 2 batches per core on 8
NeuronCores. No collectives; each core runs the same program (SPMD) on its
batch shard plus the full (replicated) weights.

Per-core dataflow (everything "transposed" so each matmul contracts over the
partition dim):
  Weights/x(0)/cond are loaded f32 with big contiguous DMAs and transposed
  128x128-block-wise ON the PE (identity matmul, ~107ns/block) with the
  f32->bf16 cast folded into the PSUM->SBUF copy. Strips interleave with the
  first projections so the PE stream is dense from ~2us (keeps HAM warm and
  avoids the serialized DMA->cast->XBAR startup chain). x(1..3) still use the
  XBAR path, hidden inside the attention steady state.
  QT = WqT.T @ xT            [co, n]
  KT = WkvT(k).T @ condT     [co, 2*l]  (both batches side by side)
  V+ones -> vaug             [l, H*(64+64)] (per batch; ones columns make the
                                             AV matmul emit row-sums too)
  sT_h = KT_h.T @ QT_h       [l, n]   (2 half-array matmuls per head pair)
  expST = Exp(sT/8 + maskbias)        (ACT, per-partition mask bias)
  ptA   = vaug_h.T @ expST_h [128, n] rows 0:64 = o~T_h, rows 64:128 = rowsum
  onormT = o~T * reciprocal_approx_fast(rowsum)   (partition-crossed DVE ops)
  y = onormT.T @ WpT         [n, co]  f32 straight to DRAM.

Emission interleaves unit u's attention with unit u+1's Q-projection so the
PE stream stays dense while ACT/DVE work on softmax.
"""

import sys

for _p in ("/opt/trn_rl_repo", "/opt/pypackages"):
    if _p not in sys.path:
        sys.path.append(_p)

import numpy as np

B = 16
N_CORES = 8
B_PER_CORE = B // N_CORES  # 2
N = 1024
C = 1024
L = 120
H = 16
DH = C // H  # 64
SCALE = DH ** -0.5  # 0.125

KC = C // 128  # 8 c-chunks of 128
HP = H // 2  # 8 head pairs
NJ = 2  # n-halves per batch
NHALF = N // NJ  # 512
NEG = -50.0  # masked-logit bias; exp(s/8 - 50) ~ 0 vs reference's -inf

_CACHE = {}


def _build_nc():
    import concourse.mybir as mybir
    import concourse.tile as tile
    from concourse import bacc, masks

    FP = mybir.dt.float32
    BF = mybir.dt.bfloat16
    I32 = mybir.dt.int32
    Exp = mybir.ActivationFunctionType.Exp
    Alu = mybir.AluOpType

    nc = bacc.Bacc("TRN2", target_bir_lowering=False, debug=False)

    x_d = nc.dram_tensor("x", [B_PER_CORE, N, C], FP, kind="ExternalInput").ap()
    cond_d = nc.dram_tensor("cond", [B_PER_CORE, L, C], FP, kind="ExternalInput").ap()
    mask_d = nc.dram_tensor("mask", [B_PER_CORE, L], I32, kind="ExternalInput").ap()
    wq_d = nc.dram_tensor("Wq", [C, C], FP, kind="ExternalInput").ap()
    wkv_d = nc.dram_tensor("Wkv", [2 * C, C], FP, kind="ExternalInput").ap()
    wp_d = nc.dram_tensor("Wp", [C, C], FP, kind="ExternalInput").ap()
    out_d = nc.dram_tensor("out", [B_PER_CORE, N, C], FP, kind="ExternalOutput").ap()

    with tile.TileContext(nc) as tc:
        with (
            tc.tile_pool(name="wt", bufs=1) as wt,
            tc.tile_pool(name="stage", bufs=3) as stage,
            tc.tile_pool(name="act", bufs=2) as act,
            tc.tile_pool(name="small", bufs=2) as small,
            tc.tile_pool(name="sm", bufs=3) as sm,
            tc.tile_pool(name="ps", bufs=8, space="PSUM") as ps,
        ):
            ident = wt.tile([128, 128], FP, tag="ident", name="ident")
            masks.make_identity(nc, ident[:])

            # ---- resident transposed weights (bf16) ----
            wqT = wt.tile([128, KC, C], BF, tag="wqT", name="wqT")
            wkvT = wt.tile([128, KC, 2 * C], BF, tag="wkvT", name="wkvT")
            wpT = wt.tile([128, KC, C], BF, tag="wpT", name="wpT")
            # both batches' cond/K side by side: cols b*128 .. b*128+L
            condT = wt.tile([128, KC, 2 * 128], BF, tag="condT", name="condT")
            ktT = wt.tile([128, KC, 2 * 128], BF, tag="ktT", name="ktT")

            def dma_strip(dram_rows, nrows=128, zero_tail=False, eng=None):
                # [nrows<=128, 1024] f32 contiguous load into a strip tile.
                # Weights ride the gpsimd DGE queue so they stream in parallel
                # with the x loads on the scalar queue.
                fst = stage.tile([128, C], FP, tag="fst", name="fst")
                if zero_tail:
                    nc.gpsimd.memset(fst[:], 0.0)
                (eng or nc.gpsimd).dma_start(out=fst[:nrows, :], in_=dram_rows)
                return fst

            def pe_transpose_strip(fst, outT, off):
                # fst [128, C] f32 -> outT[:, kc, off:off+128] bf16, 8 blocks
                # via PE identity-matmul transpose, cast on the PSUM->SBUF copy
                for half in range(2):
                    pt = ps.tile([128, 512], FP, tag="ps", name="t_ps")
                    for q in range(4):
                        kc = half * 4 + q
                        nc.tensor.transpose(
                            pt[:, q * 128 : (q + 1) * 128],
                            fst[:, kc * 128 : (kc + 1) * 128],
                            ident[:],
                        )
                    nc.any.tensor_copy(
                        out=outT[:, half * 4 : (half + 1) * 4, off : off + 128],
                        in_=pt[:].rearrange("p (a b) -> p a b", a=4),
                    )

            # ---- per-(batch, n-half) state ----
            units = [(b, j) for b in range(B_PER_CORE) for j in range(NJ)]
            xTs = {}
            qTs = {}

            def load_x_xbar(u):
                # XBAR path (DMA f32 -> cast bf16 -> dma_start_transpose)
                b, j = units[u]
                xT = act.tile([128, KC, NHALF], BF, tag="xT", name="xT")
                for s in range(2):
                    fst = stage.tile([128, 2, C], FP, tag="xfst", name="x_fst", bufs=2)
                    r0 = j * NHALF + s * 256
                    nc.scalar.dma_start(
                        out=fst[:],
                        in_=x_d[b, r0 : r0 + 256, :].rearrange(
                            "(po pi) c -> pi po c", pi=128
                        ),
                    )
                    bst = stage.tile([128, 2, C], BF, tag="xbst", name="x_bst", bufs=2)
                    # pinned to gpsimd: 'any' once put these on the same queue
                    # as the transpose PSUM->SBUF copies, head-of-line blocking
                    # the PE behind the x DMA wait
                    nc.gpsimd.tensor_copy(out=bst[:, 0, :], in_=fst[:, 0, :])
                    nc.gpsimd.tensor_copy(out=bst[:, 1, :], in_=fst[:, 1, :])
                    for i in range(2):
                        nc.sync.dma_start_transpose(
                            xT[:, :, (s * 2 + i) * 128 : (s * 2 + i + 1) * 128],
                            bst[:, i, :],
                        )
                xTs[u] = xT

            def load_x_pe(u):
                # PE-transpose path for unit 0 (startup)
                b, j = units[u]
                xT = act.tile([128, KC, NHALF], BF, tag="xT", name="xT")
                for s in range(4):
                    r0 = j * NHALF + s * 128
                    fst = dma_strip(x_d[b, r0 : r0 + 128, :], eng=nc.scalar)
                    pe_transpose_strip(fst, xT, s * 128)
                xTs[u] = xT

            def q_proj_chunk(u, m):
                # one output chunk m of QT for unit u (8 accumulating MMs)
                if m == 0:
                    qTs[u] = act.tile([128, KC, NHALF], BF, tag="qT", name="qT")
                xT, qT = xTs[u], qTs[u]
                pt = ps.tile([128, 512], FP, tag="ps", name="q_ps")
                for kc in range(KC):
                    nc.tensor.matmul(
                        pt[:],
                        lhsT=wqT[:, kc, m * 128 : (m + 1) * 128],
                        rhs=xT[:, kc, :],
                        start=(kc == 0),
                        stop=(kc == KC - 1),
                    )
                nc.any.tensor_copy(out=qT[:, m, :], in_=pt[:])

            # ---- phase A: x(0) + Wq strips, interleaved with Q-proj(0) ----
            load_x_pe(0)
            for s in range(KC):
                fst = dma_strip(wq_d[s * 128 : (s + 1) * 128, :])
                pe_transpose_strip(fst, wqT, s * 128)
                q_proj_chunk(0, s)

            # ---- cond (PE transpose) + mask ----
            mbs = []
            for b in range(B_PER_CORE):
                cfst = dma_strip(cond_d[b], nrows=L, zero_tail=True, eng=nc.sync)
                pe_transpose_strip(cfst, condT, b * 128)

                mi = small.tile([128, 1], I32, tag="mi", name="mi")
                nc.sync.dma_start(out=mi[:L, :], in_=mask_d[b][:, None])
                mb = small.tile([128, 1], FP, tag="mb", name="mb")
                nc.vector.tensor_copy(out=mb[:L, :], in_=mi[:L, :])
                nc.vector.tensor_scalar(
                    mb[:L, :], mb[:L, :], -NEG, NEG, Alu.mult, Alu.add
                )
                mbs.append(mb)

            # ---- KV projections, interleaved with Wkv strip transposes ----
            # vaug: per batch [L, H*128] bf16; head h occupies cols h*128 ..
            # h*128+64 = V_h, cols h*128+64 .. (h+1)*128 = ones (row-sum trick)
            vaugs = []
            for b in range(B_PER_CORE):
                vaug = small.tile([128, H * 128], BF, tag="vaug", name="vaug")
                nc.gpsimd.memset(vaug[:], 1.0)
                vaugs.append(vaug)

            # Wk strips 0..7; KT chunk m needs strip m + condT (both batches)
            for s in range(KC):
                fst = dma_strip(wkv_d[s * 128 : (s + 1) * 128, :])
                pe_transpose_strip(fst, wkvT, s * 128)
                pt = ps.tile([128, 512], FP, tag="ps", name="kt_ps")
                for kc in range(KC):
                    nc.tensor.matmul(
                        pt[:, :256],
                        lhsT=wkvT[:, kc, s * 128 : (s + 1) * 128],
                        rhs=condT[:, kc, :],
                        start=(kc == 0),
                        stop=(kc == KC - 1),
                    )
                nc.any.tensor_copy(out=ktT[:, s, :], in_=pt[:, :256])

            # Wv strips 8..15 + V projections into vaug's V slots
            for ch in range(2):
                for q in range(4):
                    s = KC + ch * 4 + q
                    fst = dma_strip(wkv_d[s * 128 : (s + 1) * 128, :])
                    pe_transpose_strip(fst, wkvT, s * 128)
                for b in range(B_PER_CORE):
                    pt = ps.tile([128, 512], FP, tag="ps", name="v_ps")
                    for kc in range(KC):
                        nc.tensor.matmul(
                            pt[:L, :],
                            lhsT=condT[:, kc, b * 128 : b * 128 + L],
                            rhs=wkvT[:, kc, C + ch * 512 : C + (ch + 1) * 512],
                            start=(kc == 0),
                            stop=(kc == KC - 1),
                        )
                    # scatter 8 heads' V into vaug cols [h*128+64, (h+1)*128)
                    # (ones occupy [h*128, h*128+64) so row-sums land at PSUM
                    # partitions 0:64 where reciprocal_approx_fast can read)
                    nc.any.tensor_copy(
                        out=vaugs[b][:L, :]
                        .rearrange("p (h z) -> p h z", z=128)[
                            :, ch * 8 : (ch + 1) * 8, DH : 2 * DH
                        ],
                        in_=pt[:L, :].rearrange("p (h d) -> p h d", d=DH),
                    )

            # ---- main pipeline ----
            def scores_hp(u, hp):
                # PE: sT pair (half-array each); ACT: masked exp -> bf16
                b, j = units[u]
                mb, qT = mbs[b], qTs[u]
                s0 = ps.tile([128, 512], FP, tag="ps", name="s0")
                s1 = ps.tile([128, 512], FP, tag="ps", name="s1")
                nc.tensor.matmul(
                    s0[:L, :], lhsT=ktT[0:64, hp, b * 128 : b * 128 + L],
                    rhs=qT[0:64, hp, :], start=True, stop=True,
                )
                nc.tensor.matmul(
                    s1[:L, :], lhsT=ktT[64:128, hp, b * 128 : b * 128 + L],
                    rhs=qT[64:128, hp, :], start=True, stop=True,
                )
                e0 = sm.tile([128, NHALF], BF, tag="expT", name="e0", bufs=8)
                e1 = sm.tile([128, NHALF], BF, tag="expT", name="e1", bufs=8)
                nc.scalar.activation(
                    out=e0[:L, :], in_=s0[:L, :], func=Exp, bias=mb[:L, :],
                    scale=SCALE,
                )
                nc.scalar.activation(
                    out=e1[:L, :], in_=s1[:L, :], func=Exp, bias=mb[:L, :],
                    scale=SCALE,
                )
                return e0, e1

            def av_hp(u, hp, e0, e1, onormT):
                # PE: one augmented-V matmul per head -> row-sums (rows 0:64,
                # from the ones columns) and o~T (rows 64:128); DVE normalize.
                # reciprocal_approx_fast (custom-DVE ucode) misreads at a
                # nonzero partition offset, so it always runs at offset 0 and
                # the plain tensor_mul does the partition crossing.
                b, j = units[u]
                vaug = vaugs[b]
                h0, h1 = 2 * hp, 2 * hp + 1
                ptA = ps.tile([128, 512], FP, tag="ps", name="ptA")
                ptB = ps.tile([128, 512], FP, tag="ps", name="ptB")
                nc.tensor.matmul(
                    ptA[:], lhsT=vaug[:L, h0 * 128 : (h0 + 1) * 128],
                    rhs=e0[:L, :], start=True, stop=True,
                )
                nc.tensor.matmul(
                    ptB[:], lhsT=vaug[:L, h1 * 128 : (h1 + 1) * 128],
                    rhs=e1[:L, :], start=True, stop=True,
                )
                rrA = sm.tile([128, NHALF], FP, tag="rrec", name="rrA", bufs=4)
                nc.vector.reciprocal_approx_fast(out=rrA[0:64, :], in_=ptA[0:64, :])
                nc.vector.tensor_mul(
                    out=onormT[0:64, hp, :], in0=ptA[64:128, :], in1=rrA[0:64, :]
                )
                rrB = sm.tile([128, NHALF], FP, tag="rrec", name="rrB", bufs=4)
                nc.vector.reciprocal_approx_fast(out=rrB[0:64, :], in_=ptB[0:64, :])
                nc.vector.tensor_mul(
                    out=onormT[64:128, hp, :], in0=ptB[64:128, :], in1=rrB[0:64, :]
                )

            # out-projection, one (nsub, ch) chunk-group of 8 MMs at a time so
            # it can interleave into the next unit's attention PE stream
            proj_state = {}

            def proj_group(u, onormT, g):
                b, j = units[u]
                nsub, ch = divmod(g, 2)
                if ch == 0:
                    proj_state[u] = sm.tile(
                        [128, C], FP, tag="ysb", name="ysb", bufs=3
                    )
                ysb = proj_state[u]
                pt = ps.tile([128, 512], FP, tag="ps", name="y_ps")
                for kc in range(KC):
                    nc.tensor.matmul(
                        pt[:],
                        lhsT=onormT[:, kc, nsub * 128 : (nsub + 1) * 128],
                        rhs=wpT[:, kc, ch * 512 : (ch + 1) * 512],
                        start=(kc == 0),
                        stop=(kc == KC - 1),
                    )
                nc.any.tensor_copy(out=ysb[:, ch * 512 : (ch + 1) * 512], in_=pt[:])
                if ch == 1:
                    row0 = j * NHALF + nsub * 128
                    nc.sync.dma_start(out=out_d[b, row0 : row0 + 128, :], in_=ysb[:])

            # Unit pipeline. Per unit u (PE order, all deps already on-chip):
            #   [scores hp][proj group of unit u-1 | Wp strip][av hp-1] x8,
            #   then Q(u+1). x(u+1) XBAR-loads during attn(u); proj(u)
            #   interleaves into attn(u+1); Wp transposes into attn(0).
            prev = None  # (unit, onormT) with projection still pending
            for u in range(len(units)):
                b, j = units[u]
                if u + 1 < len(units):
                    load_x_xbar(u + 1)
                onormT = act.tile([128, KC, NHALF], BF, tag="onormT", name="onormT")
                pending = None
                for hp in range(HP):
                    e0, e1 = scores_hp(u, hp)
                    if prev is not None:
                        proj_group(prev[0], prev[1], hp)
                    else:
                        # sync queue: idle until the first out store (unit 1)
                        fst = dma_strip(
                            wp_d[hp * 128 : (hp + 1) * 128, :], eng=nc.sync
                        )
                        pe_transpose_strip(fst, wpT, hp * 128)
                    if pending is not None:
                        av_hp(u, pending[0], pending[1], pending[2], onormT)
                    pending = (hp, e0, e1)
                av_hp(u, pending[0], pending[1], pending[2], onormT)
                if prev is not None:
                    qTs.pop(prev[0], None)
                xTs.pop(u, None)
                if u + 1 < len(units):
                    for m in range(KC):
                        q_proj_chunk(u + 1, m)
                prev = (u, onormT)

            # drain: projection of the last unit
            for g in range(8):
                proj_group(prev[0], prev[1], g)

    nc.compile()
    return nc


def get_nc():
    if "nc" not in _CACHE:
        _CACHE["nc"] = _build_nc()
    return _CACHE["nc"]


def make_in_maps(x, cond, mask, Wq, Wkv, Wp):
    x = np.ascontiguousarray(np.asarray(x, dtype=np.float32))
    cond = np.ascontiguousarray(np.asarray(cond, dtype=np.float32))
    mask = np.ascontiguousarray(np.asarray(mask, dtype=np.int32))
    Wq = np.ascontiguousarray(np.asarray(Wq, dtype=np.float32))
    Wkv = np.ascontiguousarray(np.asarray(Wkv, dtype=np.float32))
    Wp = np.ascontiguousarray(np.asarray(Wp, dtype=np.float32))
    in_maps = []
    for i in range(N_CORES):
        s = slice(i * B_PER_CORE, (i + 1) * B_PER_CORE)
        in_maps.append(
            {
                "x": x[s],
                "cond": cond[s],
                "mask": mask[s],
                "Wq": Wq,
                "Wkv": Wkv,
                "Wp": Wp,
            }
        )
    return in_maps


def run(x, cond, mask, Wq, Wkv, Wp, trace=False):
    from concourse import bass_utils

    nc = get_nc()
    in_maps = make_in_maps(x, cond, mask, Wq, Wkv, Wp)
    res = bass_utils.run_bass_kernel_spmd(
        nc, in_maps, core_ids=list(range(N_CORES)), trace=trace
    )
    out = np.concatenate([res.results[i]["out"] for i in range(N_CORES)], axis=0)
    return out.astype(np.float32, copy=False), res


def kernel(x, cond, mask, Wq, bq, Wkv, bkv, Wp, bp):
    # bq/bkv/bp are zeros per the problem spec (fill: zeros) and are unused.
    out, _ = run(x, cond, mask, Wq, Wkv, Wp, trace=False)
    return out


# revision 13
# speedup vs baseline: 1.0949x; 1.0949x over previous
"""Trainium2 Bass kernel for MultiHeadCrossAttention.

Problem: y = proj(softmax(mask(q @ k^T / sqrt(Dh))) @ v) with
  x: (16, 1024, 1024) f32, cond: (16, 120, 1024) f32, mask: (16, 120) i32,
  Wq: (1024, 1024), Wkv: (2048, 1024), Wp: (1024, 1024); H=16 heads, Dh=64.
  Biases are all zeros per the problem spec and are skipped.

Sharding: pure data-parallel over batch B=16 -> 2 batches per core on 8
NeuronCores. No collectives; each core runs the same program (SPMD) on its
batch shard plus the full (replicated) weights.

Per-core dataflow (everything "transposed" so each matmul contracts over the
partition dim):
  Weights/x(0)/cond are loaded f32 with big contiguous DMAs and transposed
  128x128-block-wise ON the PE (identity matmul, ~107ns/block) with the
  f32->bf16 cast folded into the PSUM->SBUF copy. Strips interleave with the
  first projections so the PE stream is dense from ~2us (keeps HAM warm and
  avoids the serialized DMA->cast->XBAR startup chain). x(1..3) still use the
  XBAR path, hidden inside the attention steady state.
  QT = WqT.T @ xT            [co, n]
  KT = WkvT(k).T @ condT     [co, 2*l]  (both batches side by side)
  V+ones -> vaug             [l, H*(64+64)] (per batch; ones columns make the
                                             AV matmul emit row-sums too)
  sT_h = KT_h.T @ QT_h       [l, n]   (2 half-array matmuls per head pair)
  expST = Exp(sT/8 + maskbias)        (ACT, per-partition mask bias)
  ptA   = vaug_h.T @ expST_h [128, n] rows 0:64 = o~T_h, rows 64:128 = rowsum
  onormT = o~T * reciprocal_approx_fast(rowsum)   (partition-crossed DVE ops)
  y = onormT.T @ WpT         [n, co]  f32 straight to DRAM.

Emission interleaves unit u's attention with unit u+1's Q-projection so the
PE stream stays dense while ACT/DVE work on softmax.
"""

import sys

for _p in ("/opt/trn_rl_repo", "/opt/pypackages"):
    if _p not in sys.path:
        sys.path.append(_p)

import numpy as np

B = 16
N_CORES = 8
B_PER_CORE = B // N_CORES  # 2
N = 1024
C = 1024
L = 120
H = 16
DH = C // H  # 64
SCALE = DH ** -0.5  # 0.125

KC = C // 128  # 8 c-chunks of 128
HP = H // 2  # 8 head pairs
NJ = 2  # n-halves per batch
NHALF = N // NJ  # 512
NEG = -50.0  # masked-logit bias; exp(s/8 - 50) ~ 0 vs reference's -inf

_CACHE = {}


def _build_nc():
    import concourse.mybir as mybir
    import concourse.tile as tile
    from concourse import bacc, masks

    FP = mybir.dt.float32
    BF = mybir.dt.bfloat16
    I32 = mybir.dt.int32
    Exp = mybir.ActivationFunctionType.Exp
    Alu = mybir.AluOpType

    nc = bacc.Bacc("TRN2", target_bir_lowering=False, debug=False)

    x_d = nc.dram_tensor("x", [B_PER_CORE, N, C], FP, kind="ExternalInput").ap()
    cond_d = nc.dram_tensor("cond", [B_PER_CORE, L, C], FP, kind="ExternalInput").ap()
    mask_d = nc.dram_tensor("mask", [B_PER_CORE, L], I32, kind="ExternalInput").ap()
    wq_d = nc.dram_tensor("Wq", [C, C], FP, kind="ExternalInput").ap()
    wkv_d = nc.dram_tensor("Wkv", [2 * C, C], FP, kind="ExternalInput").ap()
    wp_d = nc.dram_tensor("Wp", [C, C], FP, kind="ExternalInput").ap()
    out_d = nc.dram_tensor("out", [B_PER_CORE, N, C], FP, kind="ExternalOutput").ap()

    with tile.TileContext(nc) as tc:
        with (
            tc.tile_pool(name="wt", bufs=1) as wt,
            tc.tile_pool(name="stage", bufs=3) as stage,
            tc.tile_pool(name="act", bufs=2) as act,
            tc.tile_pool(name="small", bufs=2) as small,
            tc.tile_pool(name="sm", bufs=3) as sm,
            tc.tile_pool(name="ps", bufs=8, space="PSUM") as ps,
        ):
            ident = wt.tile([128, 128], FP, tag="ident", name="ident")
            masks.make_identity(nc, ident[:])

            # ---- resident transposed weights (bf16) ----
            wqT = wt.tile([128, KC, C], BF, tag="wqT", name="wqT")
            wkvT = wt.tile([128, KC, 2 * C], BF, tag="wkvT", name="wkvT")
            wpT = wt.tile([128, KC, C], BF, tag="wpT", name="wpT")
            # both batches' cond/K side by side: cols b*128 .. b*128+L
            condT = wt.tile([128, KC, 2 * 128], BF, tag="condT", name="condT")
            ktT = wt.tile([128, KC, 2 * 128], BF, tag="ktT", name="ktT")

            def dma_strip(dram_rows, nrows=128, zero_tail=False, eng=None):
                # [nrows<=128, 1024] f32 contiguous load into a strip tile.
                # Weights ride the gpsimd DGE queue so they stream in parallel
                # with the x loads on the scalar queue.
                fst = stage.tile([128, C], FP, tag="fst", name="fst", bufs=4)
                if zero_tail:
                    nc.gpsimd.memset(fst[:], 0.0)
                (eng or nc.sync).dma_start(out=fst[:nrows, :], in_=dram_rows)
                return fst

            def pe_transpose_strip(fst, outT, off):
                # fst [128, C] f32 -> outT[:, kc, off:off+128] bf16, 8 blocks
                # via PE identity-matmul transpose, cast on the PSUM->SBUF copy
                for half in range(2):
                    pt = ps.tile([128, 512], FP, tag="ps", name="t_ps")
                    for q in range(4):
                        kc = half * 4 + q
                        nc.tensor.transpose(
                            pt[:, q * 128 : (q + 1) * 128],
                            fst[:, kc * 128 : (kc + 1) * 128],
                            ident[:],
                        )
                    out_ap = outT[:, half * 4 : (half + 1) * 4, off : off + 128]
                    in_ap = pt[:].rearrange("p (a b) -> p a b", a=4)
                    if half == 0:
                        nc.vector.tensor_copy(out=out_ap, in_=in_ap)
                    else:
                        nc.scalar.copy(out=out_ap, in_=in_ap)

            # ---- per-(batch, n-half) state ----
            units = [(b, j) for b in range(B_PER_CORE) for j in range(NJ)]
            xTs = {}
            qTs = {}

            def load_x_xbar(u):
                # XBAR path (DMA f32 -> cast bf16 -> dma_start_transpose)
                b, j = units[u]
                xT = act.tile([128, KC, NHALF], BF, tag="xT", name="xT")
                for s in range(2):
                    fst = stage.tile([128, 2, C], FP, tag="xfst", name="x_fst", bufs=2)
                    r0 = j * NHALF + s * 256
                    nc.scalar.dma_start(
                        out=fst[:],
                        in_=x_d[b, r0 : r0 + 256, :].rearrange(
                            "(po pi) c -> pi po c", pi=128
                        ),
                    )
                    bst = stage.tile([128, 2, C], BF, tag="xbst", name="x_bst", bufs=2)
                    nc.vector.tensor_copy(out=bst[:, 0, :], in_=fst[:, 0, :])
                    nc.vector.tensor_copy(out=bst[:, 1, :], in_=fst[:, 1, :])
                    for i in range(2):
                        nc.sync.dma_start_transpose(
                            xT[:, :, (s * 2 + i) * 128 : (s * 2 + i + 1) * 128],
                            bst[:, i, :],
                        )
                xTs[u] = xT

            def load_x_pe(u):
                # PE-transpose path for unit 0 (startup)
                b, j = units[u]
                xT = act.tile([128, KC, NHALF], BF, tag="xT", name="xT")
                for s in range(4):
                    r0 = j * NHALF + s * 128
                    fst = dma_strip(x_d[b, r0 : r0 + 128, :], eng=nc.scalar)
                    pe_transpose_strip(fst, xT, s * 128)
                xTs[u] = xT

            def q_proj_chunk(u, m):
                # one output chunk m of QT for unit u (8 accumulating MMs)
                if m == 0:
                    qTs[u] = act.tile([128, KC, NHALF], BF, tag="qT", name="qT")
                xT, qT = xTs[u], qTs[u]
                pt = ps.tile([128, 512], FP, tag="ps", name="q_ps")
                for kc in range(KC):
                    nc.tensor.matmul(
                        pt[:],
                        lhsT=wqT[:, kc, m * 128 : (m + 1) * 128],
                        rhs=xT[:, kc, :],
                        start=(kc == 0),
                        stop=(kc == KC - 1),
                    )
                nc.any.tensor_copy(out=qT[:, m, :], in_=pt[:])

            # ---- phase A: x(0) + Wq strips, interleaved with Q-proj(0) ----
            load_x_pe(0)
            for s in range(KC):
                fst = dma_strip(wq_d[s * 128 : (s + 1) * 128, :])
                pe_transpose_strip(fst, wqT, s * 128)
                q_proj_chunk(0, s)

            # ---- cond (PE transpose) + mask ----
            mbs = []
            for b in range(B_PER_CORE):
                cfst = dma_strip(cond_d[b], nrows=L, zero_tail=True)
                pe_transpose_strip(cfst, condT, b * 128)

                mi = small.tile([128, 1], I32, tag="mi", name="mi")
                nc.sync.dma_start(out=mi[:L, :], in_=mask_d[b][:, None])
                mb = small.tile([128, 1], FP, tag="mb", name="mb")
                nc.vector.tensor_copy(out=mb[:L, :], in_=mi[:L, :])
                nc.vector.tensor_scalar(
                    mb[:L, :], mb[:L, :], -NEG, NEG, Alu.mult, Alu.add
                )
                mbs.append(mb)

            # ---- KV projections, interleaved with Wkv strip transposes ----
            # vaug: per batch [L, H*128] bf16; head h occupies cols h*128 ..
            # h*128+64 = V_h, cols h*128+64 .. (h+1)*128 = ones (row-sum trick)
            vaugs = []
            for b in range(B_PER_CORE):
                vaug = small.tile([128, H * 128], BF, tag="vaug", name="vaug")
                nc.gpsimd.memset(vaug[:], 1.0)
                vaugs.append(vaug)

            # Wk strips 0..7; KT chunk m needs strip m + condT (both batches)
            for s in range(KC):
                fst = dma_strip(wkv_d[s * 128 : (s + 1) * 128, :])
                pe_transpose_strip(fst, wkvT, s * 128)
                pt = ps.tile([128, 512], FP, tag="ps", name="kt_ps")
                for kc in range(KC):
                    nc.tensor.matmul(
                        pt[:, :256],
                        lhsT=wkvT[:, kc, s * 128 : (s + 1) * 128],
                        rhs=condT[:, kc, :],
                        start=(kc == 0),
                        stop=(kc == KC - 1),
                    )
                nc.any.tensor_copy(out=ktT[:, s, :], in_=pt[:, :256])

            # Wv strips 8..15 + V projections into vaug's V slots
            for ch in range(2):
                for q in range(4):
                    s = KC + ch * 4 + q
                    fst = dma_strip(wkv_d[s * 128 : (s + 1) * 128, :])
                    pe_transpose_strip(fst, wkvT, s * 128)
                for b in range(B_PER_CORE):
                    pt = ps.tile([128, 512], FP, tag="ps", name="v_ps")
                    for kc in range(KC):
                        nc.tensor.matmul(
                            pt[:L, :],
                            lhsT=condT[:, kc, b * 128 : b * 128 + L],
                            rhs=wkvT[:, kc, C + ch * 512 : C + (ch + 1) * 512],
                            start=(kc == 0),
                            stop=(kc == KC - 1),
                        )
                    # scatter 8 heads' V into vaug cols [h*128+64, (h+1)*128)
                    # (ones occupy [h*128, h*128+64) so row-sums land at PSUM
                    # partitions 0:64 where reciprocal_approx_fast can read)
                    nc.any.tensor_copy(
                        out=vaugs[b][:L, :]
                        .rearrange("p (h z) -> p h z", z=128)[
                            :, ch * 8 : (ch + 1) * 8, DH : 2 * DH
                        ],
                        in_=pt[:L, :].rearrange("p (h d) -> p h d", d=DH),
                    )

            # ---- main pipeline ----
            def scores_hp(u, hp):
                # PE: sT pair (half-array each); ACT: masked exp -> bf16
                b, j = units[u]
                mb, qT = mbs[b], qTs[u]
                s0 = ps.tile([128, 512], FP, tag="ps", name="s0")
                s1 = ps.tile([128, 512], FP, tag="ps", name="s1")
                nc.tensor.matmul(
                    s0[:L, :], lhsT=ktT[0:64, hp, b * 128 : b * 128 + L],
                    rhs=qT[0:64, hp, :], start=True, stop=True,
                )
                nc.tensor.matmul(
                    s1[:L, :], lhsT=ktT[64:128, hp, b * 128 : b * 128 + L],
                    rhs=qT[64:128, hp, :], start=True, stop=True,
                )
                e0 = sm.tile([128, NHALF], BF, tag="expT", name="e0", bufs=8)
                e1 = sm.tile([128, NHALF], BF, tag="expT", name="e1", bufs=8)
                nc.scalar.activation(
                    out=e0[:L, :], in_=s0[:L, :], func=Exp, bias=mb[:L, :],
                    scale=SCALE,
                )
                nc.scalar.activation(
                    out=e1[:L, :], in_=s1[:L, :], func=Exp, bias=mb[:L, :],
                    scale=SCALE,
                )
                return e0, e1

            def av_hp(u, hp, e0, e1, onormT):
                # PE: one augmented-V matmul per head -> row-sums (rows 0:64,
                # from the ones columns) and o~T (rows 64:128); DVE normalize.
                # reciprocal_approx_fast (custom-DVE ucode) misreads at a
                # nonzero partition offset, so it always runs at offset 0 and
                # the plain tensor_mul does the partition crossing.
                b, j = units[u]
                vaug = vaugs[b]
                h0, h1 = 2 * hp, 2 * hp + 1
                ptA = ps.tile([128, 512], FP, tag="ps", name="ptA")
                ptB = ps.tile([128, 512], FP, tag="ps", name="ptB")
                nc.tensor.matmul(
                    ptA[:], lhsT=vaug[:L, h0 * 128 : (h0 + 1) * 128],
                    rhs=e0[:L, :], start=True, stop=True,
                )
                nc.tensor.matmul(
                    ptB[:], lhsT=vaug[:L, h1 * 128 : (h1 + 1) * 128],
                    rhs=e1[:L, :], start=True, stop=True,
                )
                rrA = sm.tile([128, NHALF], FP, tag="rrec", name="rrA", bufs=4)
                nc.vector.reciprocal_approx_fast(out=rrA[0:64, :], in_=ptA[0:64, :])
                nc.vector.tensor_mul(
                    out=onormT[0:64, hp, :], in0=ptA[64:128, :], in1=rrA[0:64, :]
                )
                rrB = sm.tile([128, NHALF], FP, tag="rrec", name="rrB", bufs=4)
                nc.vector.reciprocal_approx_fast(out=rrB[0:64, :], in_=ptB[0:64, :])
                nc.vector.tensor_mul(
                    out=onormT[64:128, hp, :], in0=ptB[64:128, :], in1=rrB[0:64, :]
                )

            # out-projection, one (nsub, ch) chunk-group of 8 MMs at a time so
            # it can interleave into the next unit's attention PE stream
            proj_state = {}

            def proj_group(u, onormT, g):
                b, j = units[u]
                nsub, ch = divmod(g, 2)
                if ch == 0:
                    proj_state[u] = sm.tile(
                        [128, C], FP, tag="ysb", name="ysb", bufs=3
                    )
                ysb = proj_state[u]
                pt = ps.tile([128, 512], FP, tag="ps", name="y_ps")
                for kc in range(KC):
                    nc.tensor.matmul(
                        pt[:],
                        lhsT=onormT[:, kc, nsub * 128 : (nsub + 1) * 128],
                        rhs=wpT[:, kc, ch * 512 : (ch + 1) * 512],
                        start=(kc == 0),
                        stop=(kc == KC - 1),
                    )
                nc.any.tensor_copy(out=ysb[:, ch * 512 : (ch + 1) * 512], in_=pt[:])
                if ch == 1:
                    row0 = j * NHALF + nsub * 128
                    nc.sync.dma_start(out=out_d[b, row0 : row0 + 128, :], in_=ysb[:])

            # Unit pipeline. Per unit u (PE order, all deps already on-chip):
            #   [scores hp][proj group of unit u-1 | Wp strip][av hp-1] x8,
            #   then Q(u+1). x(u+1) XBAR-loads during attn(u); proj(u)
            #   interleaves into attn(u+1); Wp transposes into attn(0).
            prev = None  # (unit, onormT) with projection still pending
            for u in range(len(units)):
                b, j = units[u]
                if u + 1 < len(units):
                    load_x_xbar(u + 1)
                onormT = act.tile([128, KC, NHALF], BF, tag="onormT", name="onormT")
                pending = None
                for hp in range(HP):
                    e0, e1 = scores_hp(u, hp)
                    if prev is not None:
                        proj_group(prev[0], prev[1], hp)
                    else:
                        fst = dma_strip(wp_d[hp * 128 : (hp + 1) * 128, :])
                        pe_transpose_strip(fst, wpT, hp * 128)
                    if pending is not None:
                        av_hp(u, pending[0], pending[1], pending[2], onormT)
                    pending = (hp, e0, e1)
                av_hp(u, pending[0], pending[1], pending[2], onormT)
                if prev is not None:
                    qTs.pop(prev[0], None)
                xTs.pop(u, None)
                if u + 1 < len(units):
                    for m in range(KC):
                        q_proj_chunk(u + 1, m)
                prev = (u, onormT)

            # drain: projection of the last unit
            for g in range(8):
                proj_group(prev[0], prev[1], g)

    nc.compile()
    return nc


def get_nc():
    if "nc" not in _CACHE:
        _CACHE["nc"] = _build_nc()
    return _CACHE["nc"]


def make_in_maps(x, cond, mask, Wq, Wkv, Wp):
    x = np.ascontiguousarray(np.asarray(x, dtype=np.float32))
    cond = np.ascontiguousarray(np.asarray(cond, dtype=np.float32))
    mask = np.ascontiguousarray(np.asarray(mask, dtype=np.int32))
    Wq = np.ascontiguousarray(np.asarray(Wq, dtype=np.float32))
    Wkv = np.ascontiguousarray(np.asarray(Wkv, dtype=np.float32))
    Wp = np.ascontiguousarray(np.asarray(Wp, dtype=np.float32))
    in_maps = []
    for i in range(N_CORES):
        s = slice(i * B_PER_CORE, (i + 1) * B_PER_CORE)
        in_maps.append(
            {
                "x": x[s],
                "cond": cond[s],
                "mask": mask[s],
                "Wq": Wq,
                "Wkv": Wkv,
                "Wp": Wp,
            }
        )
    return in_maps


def run(x, cond, mask, Wq, Wkv, Wp, trace=False):
    from concourse import bass_utils

    nc = get_nc()
    in_maps = make_in_maps(x, cond, mask, Wq, Wkv, Wp)
    res = bass_utils.run_bass_kernel_spmd(
        nc, in_maps, core_ids=list(range(N_CORES)), trace=trace
    )
    out = np.concatenate([res.results[i]["out"] for i in range(N_CORES)], axis=0)
    return out.astype(np.float32, copy=False), res


def kernel(x, cond, mask, Wq, bq, Wkv, bkv, Wp, bp):
    # bq/bkv/bp are zeros per the problem spec (fill: zeros) and are unused.
    out, _ = run(x, cond, mask, Wq, Wkv, Wp, trace=False)
    return out


# revision 14
# speedup vs baseline: 1.0995x; 1.0042x over previous
"""Trainium2 Bass kernel for MultiHeadCrossAttention.

Problem: y = proj(softmax(mask(q @ k^T / sqrt(Dh))) @ v) with
  x: (16, 1024, 1024) f32, cond: (16, 120, 1024) f32, mask: (16, 120) i32,
  Wq: (1024, 1024), Wkv: (2048, 1024), Wp: (1024, 1024); H=16 heads, Dh=64.
  Biases are all zeros per the problem spec and are skipped.

Sharding: pure data-parallel over batch B=16 -> 2 batches per core on 8
NeuronCores. No collectives; each core runs the same program (SPMD) on its
batch shard plus the full (replicated) weights.

Per-core dataflow (everything "transposed" so each matmul contracts over the
partition dim):
  Weights/x(0)/cond are loaded f32 with big contiguous DMAs and transposed
  128x128-block-wise ON the PE (identity matmul, ~107ns/block) with the
  f32->bf16 cast folded into the PSUM->SBUF copy. Strips interleave with the
  first projections so the PE stream is dense from ~2us (keeps HAM warm and
  avoids the serialized DMA->cast->XBAR startup chain). x(1..3) still use the
  XBAR path, hidden inside the attention steady state.
  QT = WqT.T @ xT            [co, n]
  KT = WkvT(k).T @ condT     [co, 2*l]  (both batches side by side)
  V+ones -> vaug             [l, H*(64+64)] (per batch; ones columns make the
                                             AV matmul emit row-sums too)
  sT_h = KT_h.T @ QT_h       [l, n]   (2 half-array matmuls per head pair)
  expST = Exp(sT/8 + maskbias)        (ACT, per-partition mask bias)
  ptA   = vaug_h.T @ expST_h [128, n] rows 0:64 = o~T_h, rows 64:128 = rowsum
  onormT = o~T * reciprocal_approx_fast(rowsum)   (partition-crossed DVE ops)
  y = onormT.T @ WpT         [n, co]  f32 straight to DRAM.

Emission interleaves unit u's attention with unit u+1's Q-projection so the
PE stream stays dense while ACT/DVE work on softmax.
"""

import sys

for _p in ("/opt/trn_rl_repo", "/opt/pypackages"):
    if _p not in sys.path:
        sys.path.append(_p)

import numpy as np

B = 16
N_CORES = 8
B_PER_CORE = B // N_CORES  # 2
N = 1024
C = 1024
L = 120
H = 16
DH = C // H  # 64
SCALE = DH ** -0.5  # 0.125

KC = C // 128  # 8 c-chunks of 128
HP = H // 2  # 8 head pairs
NJ = 2  # n-halves per batch
NHALF = N // NJ  # 512
NEG = -50.0  # masked-logit bias; exp(s/8 - 50) ~ 0 vs reference's -inf

_CACHE = {}


def _build_nc():
    import concourse.mybir as mybir
    import concourse.tile as tile
    from concourse import bacc, masks

    FP = mybir.dt.float32
    BF = mybir.dt.bfloat16
    I32 = mybir.dt.int32
    Exp = mybir.ActivationFunctionType.Exp
    Alu = mybir.AluOpType

    nc = bacc.Bacc("TRN2", target_bir_lowering=False, debug=False)

    x_d = nc.dram_tensor("x", [B_PER_CORE, N, C], FP, kind="ExternalInput").ap()
    cond_d = nc.dram_tensor("cond", [B_PER_CORE, L, C], FP, kind="ExternalInput").ap()
    mask_d = nc.dram_tensor("mask", [B_PER_CORE, L], I32, kind="ExternalInput").ap()
    wq_d = nc.dram_tensor("Wq", [C, C], FP, kind="ExternalInput").ap()
    wkv_d = nc.dram_tensor("Wkv", [2 * C, C], FP, kind="ExternalInput").ap()
    wp_d = nc.dram_tensor("Wp", [C, C], FP, kind="ExternalInput").ap()
    out_d = nc.dram_tensor("out", [B_PER_CORE, N, C], FP, kind="ExternalOutput").ap()

    with tile.TileContext(nc) as tc:
        with (
            tc.tile_pool(name="wt", bufs=1) as wt,
            tc.tile_pool(name="stage", bufs=3) as stage,
            tc.tile_pool(name="act", bufs=2) as act,
            tc.tile_pool(name="small", bufs=2) as small,
            tc.tile_pool(name="sm", bufs=3) as sm,
            tc.tile_pool(name="ps", bufs=8, space="PSUM") as ps,
        ):
            ident = wt.tile([128, 128], FP, tag="ident", name="ident")
            masks.make_identity(nc, ident[:])

            # ---- HAM warm-up: ~3.5us of dummy matmuls with no DMA deps ----
            # The PE clock sits at 1.2 GHz until ~3.4us of sustained activity;
            # without this, the whole DMA-paced startup runs at half clock.
            warm = wt.tile([128, 128], BF, tag="warm", name="warm")
            nc.gpsimd.memset(warm[:], 0.0)
            for _ in range(30):
                pt = ps.tile([128, 512], FP, tag="ps", name="warm_ps")
                nc.tensor.matmul(
                    pt[:, 0:128], lhsT=warm[:], rhs=warm[:], start=True, stop=True
                )

            # ---- resident transposed weights (bf16) ----
            wqT = wt.tile([128, KC, C], BF, tag="wqT", name="wqT")
            wkvT = wt.tile([128, KC, 2 * C], BF, tag="wkvT", name="wkvT")
            wpT = wt.tile([128, KC, C], BF, tag="wpT", name="wpT")
            # both batches' cond/K side by side: cols b*128 .. b*128+L
            condT = wt.tile([128, KC, 2 * 128], BF, tag="condT", name="condT")
            ktT = wt.tile([128, KC, 2 * 128], BF, tag="ktT", name="ktT")

            def dma_strip(dram_rows, nrows=128, zero_tail=False, eng=None):
                # [nrows<=128, 1024] f32 contiguous load into a strip tile.
                # Weights ride the gpsimd DGE queue so they stream in parallel
                # with the x loads on the scalar queue.
                fst = stage.tile([128, C], FP, tag="fst", name="fst", bufs=4)
                if zero_tail:
                    nc.gpsimd.memset(fst[:], 0.0)
                (eng or nc.sync).dma_start(out=fst[:nrows, :], in_=dram_rows)
                return fst

            def pe_transpose_strip(fst, outT, off):
                # fst [128, C] f32 -> outT[:, kc, off:off+128] bf16, 8 blocks
                # via PE identity-matmul transpose, cast on the PSUM->SBUF copy
                for half in range(2):
                    pt = ps.tile([128, 512], FP, tag="ps", name="t_ps")
                    for q in range(4):
                        kc = half * 4 + q
                        nc.tensor.transpose(
                            pt[:, q * 128 : (q + 1) * 128],
                            fst[:, kc * 128 : (kc + 1) * 128],
                            ident[:],
                        )
                    out_ap = outT[:, half * 4 : (half + 1) * 4, off : off + 128]
                    in_ap = pt[:].rearrange("p (a b) -> p a b", a=4)
                    if half == 0:
                        nc.vector.tensor_copy(out=out_ap, in_=in_ap)
                    else:
                        nc.scalar.copy(out=out_ap, in_=in_ap)

            # ---- per-(batch, n-half) state ----
            units = [(b, j) for b in range(B_PER_CORE) for j in range(NJ)]
            xTs = {}
            qTs = {}

            def load_x_xbar(u):
                # XBAR path (DMA f32 -> cast bf16 -> dma_start_transpose)
                b, j = units[u]
                xT = act.tile([128, KC, NHALF], BF, tag="xT", name="xT")
                for s in range(2):
                    fst = stage.tile([128, 2, C], FP, tag="xfst", name="x_fst", bufs=2)
                    r0 = j * NHALF + s * 256
                    nc.scalar.dma_start(
                        out=fst[:],
                        in_=x_d[b, r0 : r0 + 256, :].rearrange(
                            "(po pi) c -> pi po c", pi=128
                        ),
                    )
                    bst = stage.tile([128, 2, C], BF, tag="xbst", name="x_bst", bufs=2)
                    nc.vector.tensor_copy(out=bst[:, 0, :], in_=fst[:, 0, :])
                    nc.vector.tensor_copy(out=bst[:, 1, :], in_=fst[:, 1, :])
                    for i in range(2):
                        nc.sync.dma_start_transpose(
                            xT[:, :, (s * 2 + i) * 128 : (s * 2 + i + 1) * 128],
                            bst[:, i, :],
                        )
                xTs[u] = xT

            def load_x_pe(u):
                # PE-transpose path for unit 0 (startup)
                b, j = units[u]
                xT = act.tile([128, KC, NHALF], BF, tag="xT", name="xT")
                for s in range(4):
                    r0 = j * NHALF + s * 128
                    fst = dma_strip(x_d[b, r0 : r0 + 128, :], eng=nc.scalar)
                    pe_transpose_strip(fst, xT, s * 128)
                xTs[u] = xT

            def q_proj_chunk(u, m):
                # one output chunk m of QT for unit u (8 accumulating MMs)
                if m == 0:
                    qTs[u] = act.tile([128, KC, NHALF], BF, tag="qT", name="qT")
                xT, qT = xTs[u], qTs[u]
                pt = ps.tile([128, 512], FP, tag="ps", name="q_ps")
                for kc in range(KC):
                    nc.tensor.matmul(
                        pt[:],
                        lhsT=wqT[:, kc, m * 128 : (m + 1) * 128],
                        rhs=xT[:, kc, :],
                        start=(kc == 0),
                        stop=(kc == KC - 1),
                    )
                nc.any.tensor_copy(out=qT[:, m, :], in_=pt[:])

            # ---- phase A: x(0) + Wq strips, interleaved with Q-proj(0) ----
            load_x_pe(0)
            for s in range(KC):
                fst = dma_strip(wq_d[s * 128 : (s + 1) * 128, :])
                pe_transpose_strip(fst, wqT, s * 128)
                q_proj_chunk(0, s)

            # ---- cond (PE transpose) + mask ----
            mbs = []
            for b in range(B_PER_CORE):
                cfst = dma_strip(cond_d[b], nrows=L, zero_tail=True)
                pe_transpose_strip(cfst, condT, b * 128)

                mi = small.tile([128, 1], I32, tag="mi", name="mi")
                nc.sync.dma_start(out=mi[:L, :], in_=mask_d[b][:, None])
                mb = small.tile([128, 1], FP, tag="mb", name="mb")
                nc.vector.tensor_copy(out=mb[:L, :], in_=mi[:L, :])
                nc.vector.tensor_scalar(
                    mb[:L, :], mb[:L, :], -NEG, NEG, Alu.mult, Alu.add
                )
                mbs.append(mb)

            # ---- KV projections, interleaved with Wkv strip transposes ----
            # vaug: per batch [L, H*128] bf16; head h occupies cols h*128 ..
            # h*128+64 = V_h, cols h*128+64 .. (h+1)*128 = ones (row-sum trick)
            vaugs = []
            for b in range(B_PER_CORE):
                vaug = small.tile([128, H * 128], BF, tag="vaug", name="vaug")
                nc.gpsimd.memset(vaug[:], 1.0)
                vaugs.append(vaug)

            # Wk strips 0..7; KT chunk m needs strip m + condT (both batches)
            for s in range(KC):
                fst = dma_strip(wkv_d[s * 128 : (s + 1) * 128, :])
                pe_transpose_strip(fst, wkvT, s * 128)
                pt = ps.tile([128, 512], FP, tag="ps", name="kt_ps")
                for kc in range(KC):
                    nc.tensor.matmul(
                        pt[:, :256],
                        lhsT=wkvT[:, kc, s * 128 : (s + 1) * 128],
                        rhs=condT[:, kc, :],
                        start=(kc == 0),
                        stop=(kc == KC - 1),
                    )
                nc.any.tensor_copy(out=ktT[:, s, :], in_=pt[:, :256])

            # Wv strips 8..15 + V projections into vaug's V slots
            for ch in range(2):
                for q in range(4):
                    s = KC + ch * 4 + q
                    fst = dma_strip(wkv_d[s * 128 : (s + 1) * 128, :])
                    pe_transpose_strip(fst, wkvT, s * 128)
                for b in range(B_PER_CORE):
                    pt = ps.tile([128, 512], FP, tag="ps", name="v_ps")
                    for kc in range(KC):
                        nc.tensor.matmul(
                            pt[:L, :],
                            lhsT=condT[:, kc, b * 128 : b * 128 + L],
                            rhs=wkvT[:, kc, C + ch * 512 : C + (ch + 1) * 512],
                            start=(kc == 0),
                            stop=(kc == KC - 1),
                        )
                    # scatter 8 heads' V into vaug cols [h*128+64, (h+1)*128)
                    # (ones occupy [h*128, h*128+64) so row-sums land at PSUM
                    # partitions 0:64 where reciprocal_approx_fast can read)
                    nc.any.tensor_copy(
                        out=vaugs[b][:L, :]
                        .rearrange("p (h z) -> p h z", z=128)[
                            :, ch * 8 : (ch + 1) * 8, DH : 2 * DH
                        ],
                        in_=pt[:L, :].rearrange("p (h d) -> p h d", d=DH),
                    )

            # ---- main pipeline ----
            def scores_hp(u, hp):
                # PE: sT pair (half-array each); ACT: masked exp -> bf16
                b, j = units[u]
                mb, qT = mbs[b], qTs[u]
                s0 = ps.tile([128, 512], FP, tag="ps", name="s0")
                s1 = ps.tile([128, 512], FP, tag="ps", name="s1")
                nc.tensor.matmul(
                    s0[:L, :], lhsT=ktT[0:64, hp, b * 128 : b * 128 + L],
                    rhs=qT[0:64, hp, :], start=True, stop=True,
                )
                nc.tensor.matmul(
                    s1[:L, :], lhsT=ktT[64:128, hp, b * 128 : b * 128 + L],
                    rhs=qT[64:128, hp, :], start=True, stop=True,
                )
                e0 = sm.tile([128, NHALF], BF, tag="expT", name="e0", bufs=8)
                e1 = sm.tile([128, NHALF], BF, tag="expT", name="e1", bufs=8)
                nc.scalar.activation(
                    out=e0[:L, :], in_=s0[:L, :], func=Exp, bias=mb[:L, :],
                    scale=SCALE,
                )
                nc.scalar.activation(
                    out=e1[:L, :], in_=s1[:L, :], func=Exp, bias=mb[:L, :],
                    scale=SCALE,
                )
                return e0, e1

            def av_hp(u, hp, e0, e1, onormT):
                # PE: one augmented-V matmul per head -> row-sums (rows 0:64,
                # from the ones columns) and o~T (rows 64:128); DVE normalize.
                # reciprocal_approx_fast (custom-DVE ucode) misreads at a
                # nonzero partition offset, so it always runs at offset 0 and
                # the plain tensor_mul does the partition crossing.
                b, j = units[u]
                vaug = vaugs[b]
                h0, h1 = 2 * hp, 2 * hp + 1
                ptA = ps.tile([128, 512], FP, tag="ps", name="ptA")
                ptB = ps.tile([128, 512], FP, tag="ps", name="ptB")
                nc.tensor.matmul(
                    ptA[:], lhsT=vaug[:L, h0 * 128 : (h0 + 1) * 128],
                    rhs=e0[:L, :], start=True, stop=True,
                )
                nc.tensor.matmul(
                    ptB[:], lhsT=vaug[:L, h1 * 128 : (h1 + 1) * 128],
                    rhs=e1[:L, :], start=True, stop=True,
                )
                rrA = sm.tile([128, NHALF], FP, tag="rrec", name="rrA", bufs=4)
                nc.vector.reciprocal_approx_fast(out=rrA[0:64, :], in_=ptA[0:64, :])
                nc.vector.tensor_mul(
                    out=onormT[0:64, hp, :], in0=ptA[64:128, :], in1=rrA[0:64, :]
                )
                rrB = sm.tile([128, NHALF], FP, tag="rrec", name="rrB", bufs=4)
                nc.vector.reciprocal_approx_fast(out=rrB[0:64, :], in_=ptB[0:64, :])
                nc.vector.tensor_mul(
                    out=onormT[64:128, hp, :], in0=ptB[64:128, :], in1=rrB[0:64, :]
                )

            # out-projection, one (nsub, ch) chunk-group of 8 MMs at a time so
            # it can interleave into the next unit's attention PE stream
            proj_state = {}

            def proj_group(u, onormT, g):
                b, j = units[u]
                nsub, ch = divmod(g, 2)
                if ch == 0:
                    proj_state[u] = sm.tile(
                        [128, C], FP, tag="ysb", name="ysb", bufs=3
                    )
                ysb = proj_state[u]
                pt = ps.tile([128, 512], FP, tag="ps", name="y_ps")
                for kc in range(KC):
                    nc.tensor.matmul(
                        pt[:],
                        lhsT=onormT[:, kc, nsub * 128 : (nsub + 1) * 128],
                        rhs=wpT[:, kc, ch * 512 : (ch + 1) * 512],
                        start=(kc == 0),
                        stop=(kc == KC - 1),
                    )
                nc.any.tensor_copy(out=ysb[:, ch * 512 : (ch + 1) * 512], in_=pt[:])
                if ch == 1:
                    row0 = j * NHALF + nsub * 128
                    nc.sync.dma_start(out=out_d[b, row0 : row0 + 128, :], in_=ysb[:])

            # Unit pipeline. Per unit u (PE order, all deps already on-chip):
            #   [scores hp][proj group of unit u-1 | Wp strip][av hp-1] x8,
            #   then Q(u+1). x(u+1) XBAR-loads during attn(u); proj(u)
            #   interleaves into attn(u+1); Wp transposes into attn(0).
            prev = None  # (unit, onormT) with projection still pending
            for u in range(len(units)):
                b, j = units[u]
                if u + 1 < len(units):
                    load_x_xbar(u + 1)
                onormT = act.tile([128, KC, NHALF], BF, tag="onormT", name="onormT")
                pending = None
                for hp in range(HP):
                    e0, e1 = scores_hp(u, hp)
                    if prev is not None:
                        proj_group(prev[0], prev[1], hp)
                    else:
                        fst = dma_strip(wp_d[hp * 128 : (hp + 1) * 128, :])
                        pe_transpose_strip(fst, wpT, hp * 128)
                    if pending is not None:
                        av_hp(u, pending[0], pending[1], pending[2], onormT)
                    pending = (hp, e0, e1)
                av_hp(u, pending[0], pending[1], pending[2], onormT)
                if prev is not None:
                    qTs.pop(prev[0], None)
                xTs.pop(u, None)
                if u + 1 < len(units):
                    for m in range(KC):
                        q_proj_chunk(u + 1, m)
                prev = (u, onormT)

            # drain: projection of the last unit
            for g in range(8):
                proj_group(prev[0], prev[1], g)

    nc.compile()
    return nc


def get_nc():
    if "nc" not in _CACHE:
        _CACHE["nc"] = _build_nc()
    return _CACHE["nc"]


def make_in_maps(x, cond, mask, Wq, Wkv, Wp):
    x = np.ascontiguousarray(np.asarray(x, dtype=np.float32))
    cond = np.ascontiguousarray(np.asarray(cond, dtype=np.float32))
    mask = np.ascontiguousarray(np.asarray(mask, dtype=np.int32))
    Wq = np.ascontiguousarray(np.asarray(Wq, dtype=np.float32))
    Wkv = np.ascontiguousarray(np.asarray(Wkv, dtype=np.float32))
    Wp = np.ascontiguousarray(np.asarray(Wp, dtype=np.float32))
    in_maps = []
    for i in range(N_CORES):
        s = slice(i * B_PER_CORE, (i + 1) * B_PER_CORE)
        in_maps.append(
            {
                "x": x[s],
                "cond": cond[s],
                "mask": mask[s],
                "Wq": Wq,
                "Wkv": Wkv,
                "Wp": Wp,
            }
        )
    return in_maps


def run(x, cond, mask, Wq, Wkv, Wp, trace=False):
    from concourse import bass_utils

    nc = get_nc()
    in_maps = make_in_maps(x, cond, mask, Wq, Wkv, Wp)
    res = bass_utils.run_bass_kernel_spmd(
        nc, in_maps, core_ids=list(range(N_CORES)), trace=trace
    )
    out = np.concatenate([res.results[i]["out"] for i in range(N_CORES)], axis=0)
    return out.astype(np.float32, copy=False), res


def kernel(x, cond, mask, Wq, bq, Wkv, bkv, Wp, bp):
    # bq/bkv/bp are zeros per the problem spec (fill: zeros) and are unused.
    out, _ = run(x, cond, mask, Wq, Wkv, Wp, trace=False)
    return out
